# revision 42
# baseline (speedup 1.0000x reference)
"""Trainium2 Bass kernel for nn_LogOddsPerformanceTransformer.

Computes, for each element x of Xs:
    s   = log(x) - log(1-x)              (log-odds)
    idx = clip(floor((s - bins[0]) / step), 0, NB-1)
    out = bins[idx]

The input is staged to the device as fp16 (halves input HBM traffic; x is
capped at the largest fp16 < 1 so 1-x stays positive) and the output leaves
the device as fp16 bin values (64 distinct values in [-6, 6]; cast back to
f32 on the host).  Per Ln group (ACT is the saturated engine and only does
the two table passes):
    a  = Ln(x)            b = Ln(1-x)        (ACT, fp16 out)
per chunk (pairs of chunks are emitted pass-major so consecutive DVE
instructions belong to independent chains and the engine never stalls on
its own ack latency):
    s  = a - b                               (tensor_tensor subtract)
    t1 = s*inv + (1024 + off)     off = -b0*inv - 0.5   (integer)
    t2 = clip(t1, 1024, 1024+NB-1)   # fp16 output rounding floors to grid
    t3 = (t2 - 512) - (512 - b0*inv) # exact halves -> idx + b0*inv
    out = t3 * step
The four tensor_scalar steps hit the DVE 4x fp16 mode.  Terminal passes
(t3/out) of early chunks go to Pool; outputs stream per chunk on SP HWDGE.
Data parallel over 8 NeuronCores, 524288 elements each as [128 x 4096].
"""

import sys

sys.path.insert(0, "/opt/trn_rl_repo")

from contextlib import ExitStack

import numpy as np

import concourse.bass as bass
import concourse.tile as tile
from concourse import bacc, mybir
from concourse.bass_utils import run_bass_kernel_spmd

N = 4_194_304
NCORES = 8
NPER = N // NCORES  # 524288
P = 128
F = NPER // P  # 4096

f16 = mybir.dt.float16
f32 = mybir.dt.float32
Alu = mybir.AluOpType
Act = mybir.ActivationFunctionType

# --- tunables -------------------------------------------------------------
# chunks grouped into Ln groups; chunks in one group are emitted pass-major
LN_GROUPS = ((512,), (1024,), (1024,), (512, 512), (256, 256))
# per-pass engine schedule per chunk index (flattened): d(ve) / p(ool)
SCHED = {
    "s": "ddddddd",
    "t1": "ddddddd",
    "t2": "ddddddd",
    "t3": "ddddadd",
    "o": "ppppddd",
}
# Prepared kv_writeback+trigger output path: saves ~1.6us of DMA issue latency
# in the cost model (15806ns), but the executor's SWDGE replay semantics differ
# from the cost model's and produce wrong data / hangs on the numeric path, so
# it stays disabled.  See the sync-patching block in _build.
N_TRIG = 0
TMP_BUFS = 6
# --------------------------------------------------------------------------

_BUILD_CACHE: dict[tuple, object] = {}


def _constants(bins: np.ndarray):
    """Host-side constants; None if bins don't fit the fp16 fused-floor path
    (needs uniform spacing, <= 64 bins, integer floor offset, and the unbias
    constants representable in fp16)."""
    b64 = bins.astype(np.float64)
    nb = len(bins)
    if nb > 64:
        return None
    step = np.float32((b64[-1] - b64[0]) / (nb - 1))
    inv = np.float32((nb - 1) / (b64[-1] - b64[0]))
    off = -b64[0] * float(inv) - 0.5
    uniform = np.allclose(np.diff(b64), (b64[-1] - b64[0]) / (nb - 1), rtol=0, atol=1e-5)
    C = 1024.0 + off
    HI = 1024.0 + (nb - 1)
    U2 = 512.0
    U2b = 512.0 + off + 0.5  # = 512 - b0*inv
    exact = (
        off == round(off)
        and float(np.float16(C)) == C
        and float(np.float16(U2b)) == U2b
        and abs(off) < 512
    )
    if not (uniform and exact):
        return None
    return tuple(float(v) for v in (step, inv, C, HI, U2, U2b))


def _kv_out_ap(o_d):
    """View the [128, F] dram output as kv_writeback's [1, 128, 1, F]."""
    return o_d[:].rearrange("p (a b n) -> a p b n", a=1, b=1)


def _kv_in_ap(ot):
    """View a [128, w] SBUF tile as kv_writeback's [128, 1, 1, w]."""
    return ot[:].rearrange("p (a b n) -> p a b n", a=1, b=1)


def _build(step, inv, C, HI, U2, U2b):
    chunks = [c for g in LN_GROUPS for c in g]
    assert sum(chunks) == F
    ngroups = len(LN_GROUPS)
    coff = [0]
    for c in chunks:
        coff.append(coff[-1] + c)

    nc = bacc.Bacc(
        "TRN2", target_bir_lowering=False, debug=False, num_swdge_queues=max(N_TRIG, 1)
    )
    x_d = nc.dram_tensor("x", [P, F], f16, kind="ExternalInput").ap()
    o_d = nc.dram_tensor("o", [P, F], f16, kind="ExternalOutput").ap()

    eng = {"d": nc.vector, "p": nc.gpsimd}
    trig_groups = {}  # group idx -> (queue, sem)

    with tile.TileContext(nc) as tc, ExitStack() as ctx:
        xpool = ctx.enter_context(tc.tile_pool(name="xpool", bufs=1))
        opool = ctx.enter_context(tc.tile_pool(name="opool", bufs=1))
        abpool = ctx.enter_context(tc.tile_pool(name="abpool", bufs=2))
        tmp = ctx.enter_context(tc.tile_pool(name="tmp", bufs=TMP_BUFS))
        cpool = ctx.enter_context(tc.tile_pool(name="cpool", bufs=1))

        # one input DMA per Ln group, high priority
        x_tiles = []
        with tc.high_priority():
            ci = 0
            for gi, g in enumerate(LN_GROUPS):
                lo, hi = coff[ci], coff[ci + len(g)]
                xt = xpool.tile([P, hi - lo], f16, tag=f"x{gi}", name=f"xt{gi}")
                nc.sync.dma_start(xt[:], x_d[:, lo:hi])
                x_tiles.append((xt, lo))
                ci += len(g)

        # output tiles; the last N_TRIG groups write back via SWDGE descriptors
        # prepared up front on Pool and fired by trigger_dma at completion
        # (skips HWDGE issue + DGE delay on the critical drain path)
        o_groups = []
        ci = 0
        for gi, g in enumerate(LN_GROUPS):
            lo, hi = coff[ci], coff[ci + len(g)]
            if gi >= ngroups - N_TRIG:
                # concrete SBUF tensor: the prep's descriptor must capture the
                # real address at emission, before tile pools finalize
                og = nc.alloc_sbuf_tensor(f"ogbuf{gi}", [P, hi - lo], f16).ap()
            else:
                og = opool.tile([P, hi - lo], f16, tag=f"o{gi}", name=f"og{gi}")
            o_groups.append((og, lo, hi))
            ci += len(g)
        for q, gi in enumerate(range(ngroups - N_TRIG, ngroups)):
            og, lo, hi = o_groups[gi]
            idx = cpool.tile([P, 1], mybir.dt.int32, tag=f"kvidx{gi}", name=f"kvidx{gi}")
            nc.gpsimd.memset(idx[:], lo)
            sem = nc.alloc_semaphore(f"kvdma{gi}")
            data_sem = nc.alloc_semaphore(f"kvdat{gi}")
            nc.gpsimd.kv_writeback(
                _kv_out_ap(o_d),
                _kv_in_ap(og),
                idx[:],
                prepare_only=True,
                sem=sem,
                queue_num=q,
            )
            trig_groups[gi] = (q, sem, data_sem)

        ci = 0
        for gi, g in enumerate(LN_GROUPS):
            lo, hi = coff[ci], coff[ci + len(g)]
            xt, xlo = x_tiles[gi]
            xs = xt[:, lo - xlo : hi - xlo]
            a = abpool.tile([P, hi - lo], f16, tag=f"a{gi % 2}", name=f"a{gi}")
            b = abpool.tile([P, hi - lo], f16, tag=f"b{gi % 2}", name=f"b{gi}")
            nc.scalar.activation(a[:], xs, Act.Ln)
            nc.scalar.activation(b[:], xs, Act.Ln, 1.0, -1.0)

            cids = list(range(ci, ci + len(g)))
            sl = {c: (slice(None), slice(coff[c] - lo, coff[c + 1] - lo)) for c in cids}
            ts = {}
            for c in cids:  # pass-major over the group's chunks
                ts[c] = tmp.tile([P, chunks[c]], f16, tag=f"s{c % 2}", name=f"s{c}")
                eng[SCHED["s"][c]].tensor_tensor(ts[c][:], a[sl[c]], b[sl[c]], Alu.subtract)
            t1 = {}
            for c in cids:
                t1[c] = tmp.tile([P, chunks[c]], f16, tag=f"t1{c % 2}", name=f"t1{c}")
                e = SCHED["t1"][c]
                if e == "a":
                    nc.scalar.activation(t1[c][:], ts[c][:], Act.Copy, C, inv)
                else:
                    eng[e].tensor_scalar(t1[c][:], ts[c][:], inv, C, Alu.mult, Alu.add)
            t2 = {}
            for c in cids:
                t2[c] = tmp.tile([P, chunks[c]], f16, tag=f"t2{c % 2}", name=f"t2{c}")
                eng[SCHED["t2"][c]].tensor_scalar(
                    t2[c][:], t1[c][:], 1024.0, HI, Alu.max, Alu.min
                )
            t3 = {}
            for c in cids:
                t3[c] = tmp.tile([P, chunks[c]], f16, tag=f"t3{c % 2}", name=f"t3{c}")
                e = SCHED["t3"][c]
                if e == "a":
                    # ACT's affine is f32-internal, so one fused subtract is exact
                    nc.scalar.activation(t3[c][:], t2[c][:], Act.Copy, -(U2 + U2b), 1.0)
                else:
                    eng[e].tensor_scalar(
                        t3[c][:], t2[c][:], U2, U2b, Alu.subtract, Alu.subtract
                    )
            # one output tile + DMA per Ln group
            og = o_groups[gi][0]
            o_engines = []
            for c in cids:
                e = SCHED["o"][c]
                if e == "a":
                    nc.scalar.activation(og[sl[c]], t3[c][:], Act.Copy, 0.0, step)
                else:
                    eng[e].tensor_scalar(og[sl[c]], t3[c][:], step, None, Alu.mult)
                o_engines.append(e)
            if gi in trig_groups:
                # a tiny dummy per writer engine carries the data_sem inc
                # (walrus rejects sem updates on TS instructions); same-engine
                # program order places it after that engine's o-passes
                q, _sem, data_sem = trig_groups[gi]
                wset = sorted(set(o_engines))
                for e in wset:
                    assert e in ("d", "p"), "trig group o-pass must be DVE/Pool"
                    eng[e].sem_inc(data_sem, 1)
                nc.gpsimd.wait_ge(data_sem, len(wset))
                nc.gpsimd.trigger_dma(count=None, queue_num=q)
            else:
                nc.sync.dma_start(o_d[:, lo:hi], og[:])
            ci += len(g)

    # The tile framework's accounting for prepared writebacks assumes the DMA
    # completion lands on its DMASW lane sems, while kv_writeback encodes our
    # kvdma{gi} sems into the descriptors.  Additionally it emits ring-slot /
    # WAR ordering waits *upstream* of the trigger, which would deadlock (the
    # completion requires the trigger to fire first).  Data ordering into the
    # DMA is guaranteed separately by the explicit data_sem wait_ge on the
    # trigger, and each SWDGE ring slot is used exactly once here.  So:
    # waits naming DMASW{q}/kvdma that appear BEFORE queue q's trigger are
    # dropped; waits appearing after it are redirected to kvdma{gi}.
    if trig_groups:
        # the framework's DMASW ring-slot sem resets double as doorbells that
        # would fire our prepared descriptors early with stale data: drop them
        for bb in nc.m.functions[0].blocks:
            keep = [i for i in bb.instructions if type(i).__name__ != "InstIncSwdgeSem"]
            if len(keep) != len(bb.instructions):
                bb.instructions[:] = keep
        lane_to_sem = {f"DMASW{q}": sem for q, sem, _d in trig_groups.values()}
        kv_to_q = {sem.name: q for q, sem, _d in trig_groups.values()}
        insns = [ins for bb in nc.m.functions[0].blocks for ins in bb.instructions]
        pos = {ins.name: i for i, ins in enumerate(insns)}
        trig_pos = {}
        for ins in insns:
            if type(ins).__name__ == "InstTriggerDma":
                trig_pos[ins.queue_num] = pos[ins.name]
        for ins in insns:
            si = ins.sync_info
            if not si or not si.on_wait:
                continue
            new_waits, changed = [], False
            for w in si.on_wait:
                if w.sync_type == "semaphore" and w.ant_name:
                    lane = w.ant_name.split("_")[0]
                    q = None
                    if lane in lane_to_sem:
                        q = int(lane[len("DMASW"):])
                        sem = lane_to_sem[lane]
                    elif w.ant_name in kv_to_q:
                        q = kv_to_q[w.ant_name]
                        sem = None
                    if q is not None:
                        changed = True
                        if pos[ins.name] < trig_pos[q]:
                            continue  # upstream of the trigger: circular, drop
                        if sem is not None:
                            w = mybir.SyncWait(
                                sync_type="semaphore",
                                id=sem.num,
                                ant_name=sem.name,
                                wait_mode=w.wait_mode,
                                wait_value=w.wait_value,
                                wait_reg=w.wait_reg,
                            )
                new_waits.append(w)
            if changed:
                si.on_wait = new_waits

    nc.compile()
    return nc


def build(bins: np.ndarray):
    key = _constants(bins)
    if key is None:
        raise NotImplementedError("bins not supported by the fp16 fused-floor kernel")
    if key not in _BUILD_CACHE:
        _BUILD_CACHE[key] = _build(*key)
    return _BUILD_CACHE[key]


FP16_BELOW_ONE = np.float16(1.0 - 2.0**-11)


def make_in_maps(Xs: np.ndarray):
    x16 = np.minimum(Xs.astype(np.float16), FP16_BELOW_ONE)
    shards = x16.reshape(NCORES, P, F)
    return [{"x": shards[c]} for c in range(NCORES)]


def kernel(Xs: np.ndarray, bins: np.ndarray) -> np.ndarray:
    Xs = np.asarray(Xs, dtype=np.float32)
    bins = np.asarray(bins, dtype=np.float32)
    nc = build(bins)
    res = run_bass_kernel_spmd(nc, make_in_maps(Xs), core_ids=list(range(NCORES)))
    out = np.concatenate([r["o"].reshape(-1) for r in res.results])
    return out.astype(np.float32)


# revision 44
# speedup vs baseline: 1.0214x; 1.0214x over previous
"""Trainium2 Bass kernel for nn_LogOddsPerformanceTransformer.

Computes, for each element x of Xs:
    s   = log(x) - log(1-x)              (log-odds)
    idx = clip(floor((s - bins[0]) / step), 0, NB-1)
    out = bins[idx]

The input is staged to the device as fp16 (halves input HBM traffic; x is
capped at the largest fp16 < 1 so 1-x stays positive) and the output leaves
the device as fp16 bin values (64 distinct values in [-6, 6]; cast back to
f32 on the host).  Per Ln group (ACT is the saturated engine and only does
the two table passes):
    a  = Ln(x)            b = Ln(1-x)        (ACT, fp16 out)
per chunk (pairs of chunks are emitted pass-major so consecutive DVE
instructions belong to independent chains and the engine never stalls on
its own ack latency):
    s  = a - b                               (tensor_tensor subtract)
    t1 = s*inv + (1024 + off)     off = -b0*inv - 0.5   (integer)
    t2 = clip(t1, 1024, 1024+NB-1)   # fp16 output rounding floors to grid
    t3 = (t2 - 512) - (512 - b0*inv) # exact halves -> idx + b0*inv
    out = t3 * step
The four tensor_scalar steps hit the DVE 4x fp16 mode.  Terminal passes
(t3/out) of early chunks go to Pool; outputs stream per chunk on SP HWDGE.
Data parallel over 8 NeuronCores, 524288 elements each as [128 x 4096].
"""

import sys

sys.path.insert(0, "/opt/trn_rl_repo")

from contextlib import ExitStack

import numpy as np

import concourse.bass as bass
import concourse.tile as tile
from concourse import bacc, mybir
from concourse.bass_utils import run_bass_kernel_spmd

N = 4_194_304
NCORES = 8
NPER = N // NCORES  # 524288
P = 128
F = NPER // P  # 4096

f16 = mybir.dt.float16
f32 = mybir.dt.float32
Alu = mybir.AluOpType
Act = mybir.ActivationFunctionType

# --- tunables -------------------------------------------------------------
# chunks grouped into Ln groups; chunks in one group are emitted pass-major
LN_GROUPS = ((512,), (1024,), (1024,), (512, 512), (256, 256))
# per-pass engine schedule per chunk index (flattened): d(ve) / p(ool)
SCHED = {
    "s": "ddddddd",
    "t1": "ddddddd",
    "t2": "ddddddd",
    "t3": "ddddadd",
    "o": "ppppddd",
}
# Prepared kv_writeback+trigger output path: saves ~1.6us of DMA issue latency
# in the cost model (15806ns), but the executor's SWDGE replay semantics differ
# from the cost model's and produce wrong data / hangs on the numeric path, so
# it stays disabled.  See the sync-patching block in _build.
N_TRIG = 0
# groups whose a/b Ln's are fused into one double-width activation over
# [x | 1-x] (the right half built by an early DVE pass); saves the per-
# instruction ACT init on the critical stream.  Only for groups whose
# x-tile arrives well before ACT reaches them.
FUSE_LN = (False, False, True, True, True)
Y_ENG = ("d", "d", "d", "p", "p")  # engine for each fused group's 1-x pass
TMP_BUFS = 6
# --------------------------------------------------------------------------

_BUILD_CACHE: dict[tuple, object] = {}


def _constants(bins: np.ndarray):
    """Host-side constants; None if bins don't fit the fp16 fused-floor path
    (needs uniform spacing, <= 64 bins, integer floor offset, and the unbias
    constants representable in fp16)."""
    b64 = bins.astype(np.float64)
    nb = len(bins)
    if nb > 64:
        return None
    step = np.float32((b64[-1] - b64[0]) / (nb - 1))
    inv = np.float32((nb - 1) / (b64[-1] - b64[0]))
    off = -b64[0] * float(inv) - 0.5
    uniform = np.allclose(np.diff(b64), (b64[-1] - b64[0]) / (nb - 1), rtol=0, atol=1e-5)
    C = 1024.0 + off
    HI = 1024.0 + (nb - 1)
    U2 = 512.0
    U2b = 512.0 + off + 0.5  # = 512 - b0*inv
    exact = (
        off == round(off)
        and float(np.float16(C)) == C
        and float(np.float16(U2b)) == U2b
        and abs(off) < 512
    )
    if not (uniform and exact):
        return None
    return tuple(float(v) for v in (step, inv, C, HI, U2, U2b))


def _kv_out_ap(o_d):
    """View the [128, F] dram output as kv_writeback's [1, 128, 1, F]."""
    return o_d[:].rearrange("p (a b n) -> a p b n", a=1, b=1)


def _kv_in_ap(ot):
    """View a [128, w] SBUF tile as kv_writeback's [128, 1, 1, w]."""
    return ot[:].rearrange("p (a b n) -> p a b n", a=1, b=1)


def _build(step, inv, C, HI, U2, U2b):
    chunks = [c for g in LN_GROUPS for c in g]
    assert sum(chunks) == F
    ngroups = len(LN_GROUPS)
    coff = [0]
    for c in chunks:
        coff.append(coff[-1] + c)

    nc = bacc.Bacc(
        "TRN2", target_bir_lowering=False, debug=False, num_swdge_queues=max(N_TRIG, 1)
    )
    x_d = nc.dram_tensor("x", [P, F], f16, kind="ExternalInput").ap()
    o_d = nc.dram_tensor("o", [P, F], f16, kind="ExternalOutput").ap()

    eng = {"d": nc.vector, "p": nc.gpsimd}
    trig_groups = {}  # group idx -> (queue, sem)

    with tile.TileContext(nc) as tc, ExitStack() as ctx:
        xpool = ctx.enter_context(tc.tile_pool(name="xpool", bufs=1))
        opool = ctx.enter_context(tc.tile_pool(name="opool", bufs=1))
        abpool = ctx.enter_context(tc.tile_pool(name="abpool", bufs=2))
        tmp = ctx.enter_context(tc.tile_pool(name="tmp", bufs=TMP_BUFS))
        cpool = ctx.enter_context(tc.tile_pool(name="cpool", bufs=1))

        # one input DMA per Ln group, high priority; fused groups allocate a
        # double-width tile (x left, 1-x right, filled by DVE just below)
        x_tiles = []
        with tc.high_priority():
            ci = 0
            for gi, g in enumerate(LN_GROUPS):
                lo, hi = coff[ci], coff[ci + len(g)]
                w = hi - lo
                xt = xpool.tile(
                    [P, 2 * w if FUSE_LN[gi] else w], f16, tag=f"x{gi}", name=f"xt{gi}"
                )
                nc.sync.dma_start(xt[:, :w], x_d[:, lo:hi])
                x_tiles.append((xt, lo))
                ci += len(g)
        def emit_y(gi):
            lo, hi = gspan[gi]
            w = hi - lo
            xt = x_tiles[gi][0]
            e = nc.gpsimd if Y_ENG[gi] == "p" else nc.vector
            e.tensor_scalar(xt[:, w : 2 * w], xt[:, :w], 1.0, -1.0, Alu.subtract, Alu.mult)

        gspan = []
        ci = 0
        for gi, g in enumerate(LN_GROUPS):
            gspan.append((coff[ci], coff[ci + len(g)]))
            ci += len(g)
        for gi in range(ngroups):
            if FUSE_LN[gi] and Y_ENG[gi] == "p":
                emit_y(gi)

        # output tiles; the last N_TRIG groups write back via SWDGE descriptors
        # prepared up front on Pool and fired by trigger_dma at completion
        # (skips HWDGE issue + DGE delay on the critical drain path)
        o_groups = []
        ci = 0
        for gi, g in enumerate(LN_GROUPS):
            lo, hi = coff[ci], coff[ci + len(g)]
            if gi >= ngroups - N_TRIG:
                # concrete SBUF tensor: the prep's descriptor must capture the
                # real address at emission, before tile pools finalize
                og = nc.alloc_sbuf_tensor(f"ogbuf{gi}", [P, hi - lo], f16).ap()
            else:
                og = opool.tile([P, hi - lo], f16, tag=f"o{gi}", name=f"og{gi}")
            o_groups.append((og, lo, hi))
            ci += len(g)
        for q, gi in enumerate(range(ngroups - N_TRIG, ngroups)):
            og, lo, hi = o_groups[gi]
            idx = cpool.tile([P, 1], mybir.dt.int32, tag=f"kvidx{gi}", name=f"kvidx{gi}")
            nc.gpsimd.memset(idx[:], lo)
            sem = nc.alloc_semaphore(f"kvdma{gi}")
            data_sem = nc.alloc_semaphore(f"kvdat{gi}")
            nc.gpsimd.kv_writeback(
                _kv_out_ap(o_d),
                _kv_in_ap(og),
                idx[:],
                prepare_only=True,
                sem=sem,
                queue_num=q,
            )
            trig_groups[gi] = (q, sem, data_sem)

        ci = 0
        for gi, g in enumerate(LN_GROUPS):
            if gi + 1 < ngroups and FUSE_LN[gi + 1] and Y_ENG[gi + 1] == "d":
                emit_y(gi + 1)  # next group's 1-x pass slots into DVE's gap here
            lo, hi = coff[ci], coff[ci + len(g)]
            xt, xlo = x_tiles[gi]
            w = hi - lo
            if FUSE_LN[gi]:
                ab = abpool.tile([P, 2 * w], f16, tag=f"a{gi % 2}", name=f"ab{gi}")
                nc.scalar.activation(ab[:], xt[:], Act.Ln)
                a, b = ab[:, :w], ab[:, w : 2 * w]
            else:
                at = abpool.tile([P, w], f16, tag=f"a{gi % 2}", name=f"a{gi}")
                bt = abpool.tile([P, w], f16, tag=f"b{gi % 2}", name=f"b{gi}")
                nc.scalar.activation(at[:], xt[:, lo - xlo : hi - xlo], Act.Ln)
                nc.scalar.activation(bt[:], xt[:, lo - xlo : hi - xlo], Act.Ln, 1.0, -1.0)
                a, b = at[:], bt[:]

            cids = list(range(ci, ci + len(g)))
            sl = {c: (slice(None), slice(coff[c] - lo, coff[c + 1] - lo)) for c in cids}
            ts = {}
            for c in cids:  # pass-major over the group's chunks
                ts[c] = tmp.tile([P, chunks[c]], f16, tag=f"s{c % 2}", name=f"s{c}")
                eng[SCHED["s"][c]].tensor_tensor(
                    ts[c][:], a[sl[c]], b[sl[c]], Alu.subtract
                )
            t1 = {}
            for c in cids:
                t1[c] = tmp.tile([P, chunks[c]], f16, tag=f"t1{c % 2}", name=f"t1{c}")
                e = SCHED["t1"][c]
                if e == "a":
                    nc.scalar.activation(t1[c][:], ts[c][:], Act.Copy, C, inv)
                else:
                    eng[e].tensor_scalar(t1[c][:], ts[c][:], inv, C, Alu.mult, Alu.add)
            t2 = {}
            for c in cids:
                t2[c] = tmp.tile([P, chunks[c]], f16, tag=f"t2{c % 2}", name=f"t2{c}")
                eng[SCHED["t2"][c]].tensor_scalar(
                    t2[c][:], t1[c][:], 1024.0, HI, Alu.max, Alu.min
                )
            t3 = {}
            for c in cids:
                t3[c] = tmp.tile([P, chunks[c]], f16, tag=f"t3{c % 2}", name=f"t3{c}")
                e = SCHED["t3"][c]
                if e == "a":
                    # ACT's affine is f32-internal, so one fused subtract is exact
                    nc.scalar.activation(t3[c][:], t2[c][:], Act.Copy, -(U2 + U2b), 1.0)
                else:
                    eng[e].tensor_scalar(
                        t3[c][:], t2[c][:], U2, U2b, Alu.subtract, Alu.subtract
                    )
            # one output tile + DMA per Ln group
            og = o_groups[gi][0]
            o_engines = []
            for c in cids:
                e = SCHED["o"][c]
                if e == "a":
                    nc.scalar.activation(og[sl[c]], t3[c][:], Act.Copy, 0.0, step)
                else:
                    eng[e].tensor_scalar(og[sl[c]], t3[c][:], step, None, Alu.mult)
                o_engines.append(e)
            if gi in trig_groups:
                # a tiny dummy per writer engine carries the data_sem inc
                # (walrus rejects sem updates on TS instructions); same-engine
                # program order places it after that engine's o-passes
                q, _sem, data_sem = trig_groups[gi]
                wset = sorted(set(o_engines))
                for e in wset:
                    assert e in ("d", "p"), "trig group o-pass must be DVE/Pool"
                    eng[e].sem_inc(data_sem, 1)
                nc.gpsimd.wait_ge(data_sem, len(wset))
                nc.gpsimd.trigger_dma(count=None, queue_num=q)
            else:
                nc.sync.dma_start(o_d[:, lo:hi], og[:])
            ci += len(g)

    # The tile framework's accounting for prepared writebacks assumes the DMA
    # completion lands on its DMASW lane sems, while kv_writeback encodes our
    # kvdma{gi} sems into the descriptors.  Additionally it emits ring-slot /
    # WAR ordering waits *upstream* of the trigger, which would deadlock (the
    # completion requires the trigger to fire first).  Data ordering into the
    # DMA is guaranteed separately by the explicit data_sem wait_ge on the
    # trigger, and each SWDGE ring slot is used exactly once here.  So:
    # waits naming DMASW{q}/kvdma that appear BEFORE queue q's trigger are
    # dropped; waits appearing after it are redirected to kvdma{gi}.
    if trig_groups:
        # the framework's DMASW ring-slot sem resets double as doorbells that
        # would fire our prepared descriptors early with stale data: drop them
        for bb in nc.m.functions[0].blocks:
            keep = [i for i in bb.instructions if type(i).__name__ != "InstIncSwdgeSem"]
            if len(keep) != len(bb.instructions):
                bb.instructions[:] = keep
        lane_to_sem = {f"DMASW{q}": sem for q, sem, _d in trig_groups.values()}
        kv_to_q = {sem.name: q for q, sem, _d in trig_groups.values()}
        insns = [ins for bb in nc.m.functions[0].blocks for ins in bb.instructions]
        pos = {ins.name: i for i, ins in enumerate(insns)}
        trig_pos = {}
        for ins in insns:
            if type(ins).__name__ == "InstTriggerDma":
                trig_pos[ins.queue_num] = pos[ins.name]
        for ins in insns:
            si = ins.sync_info
            if not si or not si.on_wait:
                continue
            new_waits, changed = [], False
            for w in si.on_wait:
                if w.sync_type == "semaphore" and w.ant_name:
                    lane = w.ant_name.split("_")[0]
                    q = None
                    if lane in lane_to_sem:
                        q = int(lane[len("DMASW"):])
                        sem = lane_to_sem[lane]
                    elif w.ant_name in kv_to_q:
                        q = kv_to_q[w.ant_name]
                        sem = None
                    if q is not None:
                        changed = True
                        if pos[ins.name] < trig_pos[q]:
                            continue  # upstream of the trigger: circular, drop
                        if sem is not None:
                            w = mybir.SyncWait(
                                sync_type="semaphore",
                                id=sem.num,
                                ant_name=sem.name,
                                wait_mode=w.wait_mode,
                                wait_value=w.wait_value,
                                wait_reg=w.wait_reg,
                            )
                new_waits.append(w)
            if changed:
                si.on_wait = new_waits

    nc.compile()
    return nc


def build(bins: np.ndarray):
    key = _constants(bins)
    if key is None:
        raise NotImplementedError("bins not supported by the fp16 fused-floor kernel")
    if key not in _BUILD_CACHE:
        _BUILD_CACHE[key] = _build(*key)
    return _BUILD_CACHE[key]


FP16_BELOW_ONE = np.float16(1.0 - 2.0**-11)


def make_in_maps(Xs: np.ndarray):
    x16 = np.minimum(Xs.astype(np.float16), FP16_BELOW_ONE)
    shards = x16.reshape(NCORES, P, F)
    return [{"x": shards[c]} for c in range(NCORES)]


def kernel(Xs: np.ndarray, bins: np.ndarray) -> np.ndarray:
    Xs = np.asarray(Xs, dtype=np.float32)
    bins = np.asarray(bins, dtype=np.float32)
    nc = build(bins)
    res = run_bass_kernel_spmd(nc, make_in_maps(Xs), core_ids=list(range(NCORES)))
    out = np.concatenate([r["o"].reshape(-1) for r in res.results])
    return out.astype(np.float32)
